# revision 1
# baseline (speedup 1.0000x reference)
"""AttnBlock (GroupNorm + 1x1-conv QKV + spatial attention w/ softmax over
query-h + out-proj + residual) for Trainium2, 8 NeuronCores.

Sharding: core = 2*b + w_half  (4 samples x 2 halves of the w axis).
Attention here softmax-normalizes over the h index of the *query* location,
so for a fixed w column the 64 h-values form one softmax group; splitting the
spatial grid by w keeps every softmax group on a single core.

Device layout notes (per core):
  - spatial index packed w-major: p = w'*64 + h   (w' in [0,32) local half)
  - xf  [256,4096]: full sample, natural (c, h*64+w) layout (GN stats, K, V)
  - xh  [256,2048]: this half's columns, w-major packed (Q, residual)
  - S^T tile [r_keys=128, p=1024] = K^T Q, exp on ScalarE (scale=1/16 folded),
    segmented sum over h (contiguous 64-blocks) on VectorE, reciprocal,
    broadcast-multiply -> attn, then O = V^T-weighted sum via PE accumulation.
  - GroupNorm is folded into the q/k/v conv weights on device:
    xn = scale_c * x + shift_c  =>  Weff = W*diag(scale), beff = W@shift + b.
  - All big matmuls use float32r (FP22 multiply, fp32 accumulate): 1 cycle/row
    on the PE when the moving dim >= 256 (plain fp32 is 4 cycles/row).
"""

import numpy as np

import concourse.bass as bass
import concourse.bacc as bacc
import concourse.mybir as mybir
import concourse.tile as tile
from concourse.bass_utils import run_bass_kernel_spmd

B, C, H, W = 4, 256, 64, 64
N = H * W            # 4096 spatial
NH = N // 2          # 2048 per w-half
WH = W // 2          # 32 local w' values
GROUPS = 32
EPS = 1e-5
F32 = mybir.dt.float32
F32R = mybir.dt.float32r
AF = mybir.ActivationFunctionType
ALU = mybir.AluOpType
AX = mybir.AxisListType


def _r(ap):
    return ap.bitcast(F32R)


def _bcast_inner(ap, n):
    """[p, m] AP -> [p, m, n] AP with innermost step 0 (free-dim broadcast)."""
    return bass.AP(tensor=ap.tensor, offset=ap.offset, ap=[*ap.ap, [0, n]])


import os


def build_nc():
    nc = bacc.Bacc("TRN2", target_bir_lowering=False, debug=False)

    xf_d = nc.dram_tensor("xf", [C, N], F32, kind="ExternalInput")
    xh_d = nc.dram_tensor("xh", [C, NH], F32, kind="ExternalInput")
    wT_d = {t: nc.dram_tensor(f"w{t}T", [C, C], F32, kind="ExternalInput")
            for t in "qkvo"}
    brow_d = {"v": nc.dram_tensor("bv_row", [1, C], F32, kind="ExternalInput")}
    bcol_d = {t: nc.dram_tensor(f"b{t}_col", [C, 1], F32, kind="ExternalInput")
              for t in "qko"}
    gamma_d = nc.dram_tensor("gamma_c", [C, 1], F32, kind="ExternalInput")
    beta_d = nc.dram_tensor("beta_c", [C, 1], F32, kind="ExternalInput")
    g1_d = nc.dram_tensor("G1", [C, GROUPS], F32, kind="ExternalInput")
    g2_d = nc.dram_tensor("G2", [GROUPS, C], F32, kind="ExternalInput")
    ones_d = nc.dram_tensor("ones_row", [1, 512], F32, kind="ExternalInput")
    out_d = nc.dram_tensor("out", [C, NH], F32, kind="ExternalOutput")

    with tile.TileContext(nc) as tc:
        with (
            tc.tile_pool(name="persist", bufs=1) as pp,
            tc.tile_pool(name="mm", bufs=6, space="PSUM") as pmm,
            tc.tile_pool(name="opsum", bufs=2, space="PSUM") as pop,
            tc.tile_pool(name="epool", bufs=8) as pe_pool,
            tc.tile_pool(name="dpool", bufs=12) as pd_pool,
            tc.tile_pool(name="outpool", bufs=3) as pout,
        ):
            def ptile(shape, tag, dtype=F32):
                return pp.tile(shape, dtype, tag=tag, name=tag)

            def psum_t(tag_name):
                return pmm.tile([128, 512], F32, tag="mm", name=tag_name)

            # ---------------- loads ----------------
            xf = []
            xh = []
            wT = {t: [] for t in "qkvo"}
            gam, bet, g1 = [], [], []
            for i in range(2):
                t = ptile([128, N], f"xf{i}", F32R)
                for ch in range(4):
                    nc.sync.dma_start(
                        out=t[:, 1024 * ch:1024 * (ch + 1)],
                        in_=xf_d[128 * i:128 * (i + 1),
                                 1024 * ch:1024 * (ch + 1)].bitcast(F32R))
                xf.append(t)
                t = ptile([128, NH], f"xh{i}", F32R)
                for ch in range(2):
                    nc.sync.dma_start(
                        out=t[:, 1024 * ch:1024 * (ch + 1)],
                        in_=xh_d[128 * i:128 * (i + 1),
                                 1024 * ch:1024 * (ch + 1)].bitcast(F32R))
                xh.append(t)
                for w in "qkvo":
                    t = ptile([128, C], f"w{w}T{i}", F32R)
                    nc.sync.dma_start(out=t, in_=wT_d[w][128 * i:128 * (i + 1), :].bitcast(F32R))
                    wT[w].append(t)
                t = ptile([128, 1], f"gam{i}")
                nc.sync.dma_start(out=t, in_=gamma_d[128 * i:128 * (i + 1), :])
                gam.append(t)
                t = ptile([128, 1], f"bet{i}")
                nc.sync.dma_start(out=t, in_=beta_d[128 * i:128 * (i + 1), :])
                bet.append(t)
                t = ptile([128, GROUPS], f"g1_{i}")
                nc.sync.dma_start(out=t, in_=g1_d[128 * i:128 * (i + 1), :])
                g1.append(t)
            g2 = ptile([GROUPS, C], "g2")
            nc.sync.dma_start(out=g2, in_=g2_d[:, :])
            ones = ptile([1, 512], "ones", F32R)
            nc.sync.dma_start(out=ones, in_=ones_d[:, :].bitcast(F32R))
            brow = {}
            for w in "v":
                brow[w] = ptile([1, C], f"b{w}row", F32R)
                nc.sync.dma_start(out=brow[w], in_=brow_d[w][:, :].bitcast(F32R))
            bcol = {}
            for w in "qko":
                bcol[w] = []
                for i in range(2):
                    t = ptile([128, 1], f"b{w}col{i}")
                    nc.sync.dma_start(out=t, in_=bcol_d[w][128 * i:128 * (i + 1), :])
                    bcol[w].append(t)

            # ---------------- GroupNorm stats -> per-channel scale/shift ----
            NSUB = N // 512
            mstat = []
            for i in range(2):
                stats = pd_pool.tile([128, NSUB, 6], F32, tag="gnstats",
                                     name=f"gnstats{i}")
                for s in range(NSUB):
                    nc.vector.bn_stats(out=stats[:, s, :],
                                       in_=xf[i][:, 512 * s:512 * (s + 1)].bitcast(F32))
                mv = pd_pool.tile([128, 2], F32, tag="gnmv", name=f"gnmv{i}")
                nc.vector.bn_aggr(out=mv, in_=stats)
                ms = ptile([128, 2], f"mstat{i}")
                # ms = [mean_c, E[x^2]_c]
                nc.vector.tensor_mul(out=ms[:, 1:2], in0=mv[:, 0:1], in1=mv[:, 0:1])
                nc.vector.tensor_add(out=ms[:, 1:2], in0=ms[:, 1:2], in1=mv[:, 1:2])
                nc.vector.tensor_copy(out=ms[:, 0:1], in_=mv[:, 0:1])
                mstat.append(ms)

            pg_t = psum_t("pg")
            pg = pg_t[:GROUPS, :2]
            for i in range(2):
                nc.tensor.matmul(pg, lhsT=g1[i], rhs=mstat[i],
                                 start=(i == 0), stop=(i == 1))
            gstat = ptile([GROUPS, 2], "gstat")
            nc.vector.tensor_scalar_mul(out=gstat, in0=pg, scalar1=1.0 / 8.0)
            var32 = ptile([GROUPS, 1], "var32")
            nc.vector.tensor_mul(out=var32, in0=gstat[:, 0:1], in1=gstat[:, 0:1])
            nc.vector.tensor_sub(out=var32, in0=gstat[:, 1:2], in1=var32)
            std32 = ptile([GROUPS, 1], "std32")
            eps_t = ptile([GROUPS, 1], "eps_t")
            nc.vector.memset(eps_t, EPS)
            nc.scalar.activation(out=std32, in_=var32, func=AF.Sqrt, bias=eps_t)
            rstd = ptile([GROUPS, 1], "rstd")
            nc.vector.reciprocal(out=rstd, in_=std32)
            # one Newton polish of rsqrt: y <- y*(1.5 - 0.5*(var+eps)*y^2)
            tnr = ptile([GROUPS, 1], "tnr")
            nc.vector.tensor_mul(out=tnr, in0=rstd, in1=rstd)
            nc.vector.tensor_mul(out=tnr, in0=tnr, in1=var32)
            vepsy = ptile([GROUPS, 1], "vepsy")
            nc.vector.tensor_mul(out=vepsy, in0=rstd, in1=rstd)
            nc.vector.tensor_scalar_mul(out=vepsy, in0=vepsy, scalar1=EPS)
            nc.vector.tensor_add(out=tnr, in0=tnr, in1=vepsy)
            nc.vector.tensor_scalar_mul(out=tnr, in0=tnr, scalar1=-0.5)
            nc.vector.tensor_scalar_add(out=tnr, in0=tnr, scalar1=1.5)
            nc.vector.tensor_mul(out=rstd, in0=rstd, in1=tnr)

            grstat = ptile([GROUPS, 2], "grstat")
            nc.vector.tensor_copy(out=grstat[:, 0:1], in_=gstat[:, 0:1])
            nc.vector.tensor_copy(out=grstat[:, 1:2], in_=rstd)

            sc, sh = [], []
            for i in range(2):
                pc_t = psum_t(f"pc{i}")
                pc = pc_t[:128, :2]
                nc.tensor.matmul(pc, lhsT=g2[:, 128 * i:128 * (i + 1)],
                                 rhs=grstat, start=True, stop=True)
                s = ptile([128, 1], f"sc{i}")
                nc.vector.tensor_mul(out=s, in0=pc[:, 1:2], in1=gam[i])
                sc.append(s)
                h = ptile([128, 1], f"sh{i}", F32R)
                nc.vector.tensor_mul(out=h, in0=pc[:, 0:1], in1=s)
                nc.vector.tensor_sub(out=h, in0=bet[i], in1=h)
                sh.append(h)

            # effective v bias as a row (per-free-column bias for V^T)
            beffr = {}
            for w in "v":
                rp_t = psum_t(f"br{w}")
                rp = rp_t[:1, :C]
                for i in range(2):
                    nc.tensor.matmul(rp, lhsT=sh[i], rhs=wT[w][i],
                                     start=(i == 0), stop=(i == 1))
                bt = ptile([1, C], f"beff{w}", F32R)
                nc.vector.tensor_add(out=bt, in0=rp, in1=brow[w])
                beffr[w] = bt
            # effective q,k biases as columns (per-partition bias for ACT fuse)
            beffc = {}
            for w in "qk":
                beffc[w] = []
                for j in range(2):
                    bp_t = psum_t(f"bc{w}{j}")
                    bp = bp_t[:128, :1]
                    for i in range(2):
                        nc.tensor.matmul(bp,
                                         lhsT=wT[w][i][:, 128 * j:128 * (j + 1)].bitcast(F32),
                                         rhs=sh[i].bitcast(F32),
                                         start=(i == 0), stop=(i == 1))
                    t = ptile([128, 1], f"beffc{w}{j}")
                    nc.vector.tensor_add(out=t, in0=bp, in1=bcol[w][j])
                    beffc[w].append(t)

            # scale conv weights in place: WeffT[i,o] = wT[i,o] * scale_i
            for w in "qkv":
                for i in range(2):
                    nc.vector.tensor_scalar_mul(out=wT[w][i], in0=wT[w][i],
                                                scalar1=sc[i])

            # ---------------- convs: K, Q(half), V^T ----------------
            k_sb = [ptile([128, N], "k0", F32R), ptile([128, N], "k1", F32R)]
            q_sb = [ptile([128, NH], "q0", F32R), ptile([128, NH], "q1", F32R)]
            for j in range(2):
                for s in range(N // 512):
                    kp = psum_t(f"kp{j}_{s}")[:, :512]
                    for i in range(2):
                        nc.tensor.matmul(kp,
                                         lhsT=_r(wT["k"][i][:, 128 * j:128 * (j + 1)]),
                                         rhs=_r(xf[i][:, 512 * s:512 * (s + 1)]),
                                         start=(i == 0), stop=(i == 1))
                    nc.scalar.activation(out=k_sb[j][:, 512 * s:512 * (s + 1)],
                                         in_=kp, func=AF.Identity,
                                         bias=beffc["k"][j])
            for j in range(2):
                for s in range(NH // 512):
                    qp = psum_t(f"qp{j}_{s}")[:, :512]
                    for i in range(2):
                        nc.tensor.matmul(qp,
                                         lhsT=_r(wT["q"][i][:, 128 * j:128 * (j + 1)]),
                                         rhs=_r(xh[i][:, 512 * s:512 * (s + 1)]),
                                         start=(i == 0), stop=(i == 1))
                    nc.scalar.activation(out=q_sb[j][:, 512 * s:512 * (s + 1)],
                                         in_=qp, func=AF.Identity,
                                         bias=beffc["q"][j])

            vT = []
            for rt in range(N // 128):
                vp = psum_t(f"vp{rt}")[:, :C]
                for i in range(2):
                    nc.tensor.matmul(vp,
                                     lhsT=_r(xf[i][:, 128 * rt:128 * (rt + 1)]),
                                     rhs=_r(wT["v"][i]),
                                     start=(i == 0), stop=False)
                nc.tensor.matmul(vp, lhsT=_r(ones[:, :128]), rhs=_r(beffr["v"]),
                                 start=False, stop=True)
                t = ptile([128, C], f"vT{rt}", F32R)
                nc.vector.tensor_copy(out=t, in_=vp)
                vT.append(t)

            conv_only = os.environ.get("KSTAGE", "full") == "conv"
            if conv_only:
                for ct in range(2):
                    dbg = pout.tile([128, NH], F32, tag="dbg", name=f"dbg{ct}", bufs=2)
                    nc.vector.tensor_copy(out=dbg, in_=k_sb[ct][:, :NH].bitcast(F32))
                    nc.sync.dma_start(out=out_d[128 * ct:128 * (ct + 1), :], in_=dbg)
            if not conv_only:
              o_sb = [ptile([128, NH], "o_sb0", F32R), ptile([128, NH], "o_sb1", F32R)]

              # ---------------- attention ----------------
              for qt in range(4):
                  o_ps = [pop.tile([128, 512], F32, tag="o", name=f"ops{qt}_{ct}")
                          for ct in range(2)]
                  qcols = slice(512 * qt, 512 * (qt + 1))
                  for rt in range(N // 128):
                      e_t = pe_pool.tile([128, 512], F32R, tag="e",
                                         name=f"e{qt}_{rt}")
                      spx = psum_t(f"sp{qt}_{rt}")
                      for i in range(2):
                          nc.tensor.matmul(
                              spx,
                              lhsT=_r(k_sb[i][:, 128 * rt:128 * (rt + 1)]),
                              rhs=_r(q_sb[i][:, qcols]),
                              start=(i == 0), stop=(i == 1))
                      # E = exp(S/16); PSUM source, SBUF dest
                      nc.scalar.activation(out=e_t, in_=spx,
                                           func=AF.Exp, scale=1.0 / 16.0)
                      e3 = e_t.rearrange("p (w h) -> p w h", h=64)
                      d_t = pd_pool.tile([128, 8], F32, tag="d",
                                         name=f"d{qt}_{rt}")
                      nc.vector.tensor_reduce(out=d_t, in_=e3, axis=AX.X,
                                              op=ALU.add)
                      r_t = pd_pool.tile([128, 8], F32, tag="r",
                                         name=f"r{qt}_{rt}")
                      nc.vector.reciprocal(out=r_t, in_=d_t)
                      mul_eng = nc.vector if qt >= 1 and (rt + qt) % 4 == 0 else nc.gpsimd
                      mul_eng.tensor_mul(out=e3, in0=e3,
                                         in1=_bcast_inner(r_t, 64))
                      for ct in range(2):
                          nc.tensor.matmul(
                              o_ps[ct],
                              lhsT=_r(vT[rt][:, 128 * ct:128 * (ct + 1)]),
                              rhs=_r(e_t),
                              start=(rt == 0), stop=(rt == N // 128 - 1))
                  for ct in range(2):
                      nc.scalar.copy(out=o_sb[ct][:, qcols], in_=o_ps[ct])

                  # ---------------- out-proj + residual for this quarter -----
                  for ct in range(2):
                      prj = psum_t(f"prj{qt}_{ct}")
                      for i in range(2):
                          nc.tensor.matmul(
                              prj,
                              lhsT=_r(wT["o"][i][:, 128 * ct:128 * (ct + 1)]),
                              rhs=_r(o_sb[i][:, qcols]),
                              start=(i == 0), stop=(i == 1))
                      ot = pout.tile([128, 512], F32, tag="ot",
                                     name=f"ot{qt}_{ct}")
                      nc.vector.scalar_tensor_tensor(
                          out=ot, in0=prj, scalar=bcol["o"][ct],
                          in1=xh[ct][:, qcols].bitcast(F32),
                          op0=ALU.add, op1=ALU.add)
                      nc.sync.dma_start(out=out_d[128 * ct:128 * (ct + 1), qcols],
                                        in_=ot)
    nc.compile()
    return nc


_NC = None


def _get_nc():
    global _NC
    if _NC is None:
        _NC = build_nc()
    return _NC


def _prep_in_maps(x, gamma, beta, q_w, q_b, k_w, k_b, v_w, v_b, o_w, o_b):
    x = np.ascontiguousarray(np.asarray(x, np.float32))
    g1 = np.zeros((C, GROUPS), np.float32)
    g1[np.arange(C), np.arange(C) // (C // GROUPS)] = 1.0
    shared = {
        "gamma_c": np.asarray(gamma, np.float32).reshape(C, 1).copy(),
        "beta_c": np.asarray(beta, np.float32).reshape(C, 1).copy(),
        "G1": g1,
        "G2": np.ascontiguousarray(g1.T),
        "ones_row": np.ones((1, 512), np.float32),
    }
    for t, wm, bv in (("q", q_w, q_b), ("k", k_w, k_b),
                      ("v", v_w, v_b), ("o", o_w, o_b)):
        shared[f"w{t}T"] = np.ascontiguousarray(np.asarray(wm, np.float32).T)
        if t == "v":
            shared["bv_row"] = np.asarray(bv, np.float32).reshape(1, C).copy()
        else:
            shared[f"b{t}_col"] = np.asarray(bv, np.float32).reshape(C, 1).copy()
    in_maps = []
    for core in range(8):
        b, half = core // 2, core % 2
        xb = x[b].reshape(C, N)
        xh = np.ascontiguousarray(
            x[b][:, :, half * WH:(half + 1) * WH].transpose(0, 2, 1)
        ).reshape(C, NH)
        in_maps.append(dict(shared, xf=np.ascontiguousarray(xb), xh=xh))
    return in_maps


def run(trace=False, **inputs):
    in_maps = _prep_in_maps(**inputs)
    nc = _get_nc()
    res = run_bass_kernel_spmd(nc, in_maps, core_ids=list(range(8)), trace=trace)
    x = np.asarray(inputs["x"], np.float32)
    out = np.empty((B, C, H, W), np.float32)
    for core in range(8):
        b, half = core // 2, core % 2
        oh = res.results[core]["out"].reshape(C, WH, H).transpose(0, 2, 1)
        out[b][:, :, half * WH:(half + 1) * WH] = oh
    return out, res


def kernel(**inputs):
    out, _ = run(trace=False, **inputs)
    return out



# revision 5
# speedup vs baseline: 26.4313x; 26.4313x over previous
"""AttnBlock (GroupNorm + 1x1-conv QKV + spatial attention w/ softmax over
query-h + out-proj + residual) for Trainium2, 8 NeuronCores.

Sharding: core = 2*b + w_half  (4 samples x 2 halves of the w axis).
Attention here softmax-normalizes over the h index of the *query* location,
so for a fixed w column the 64 h-values form one softmax group; splitting the
spatial grid by w keeps every softmax group on a single core.

Device layout notes (per core):
  - spatial index packed w-major: p = w'*64 + h   (w' in [0,32) local half)
  - xf  [256,4096]: full sample, natural (c, h*64+w) layout (GN stats, K, V)
  - xh  [256,2048]: this half's columns, w-major packed (Q, residual)
  - S^T tile [r_keys=128, p=1024] = K^T Q, exp on ScalarE (scale=1/16 folded),
    segmented sum over h (contiguous 64-blocks) on VectorE, reciprocal,
    broadcast-multiply -> attn, then O = V^T-weighted sum via PE accumulation.
  - GroupNorm is folded into the q/k/v conv weights on device:
    xn = scale_c * x + shift_c  =>  Weff = W*diag(scale), beff = W@shift + b.
  - All big matmuls use float32r (FP22 multiply, fp32 accumulate): 1 cycle/row
    on the PE when the moving dim >= 256 (plain fp32 is 4 cycles/row).
"""

import numpy as np

import concourse.bass as bass
import concourse.bacc as bacc
import concourse.mybir as mybir
import concourse.tile as tile
from concourse.bass_utils import run_bass_kernel_spmd

B, C, H, W = 4, 256, 64, 64
N = H * W            # 4096 spatial
NH = N // 2          # 2048 per w-half
WH = W // 2          # 32 local w' values
GROUPS = 32
EPS = 1e-5
F32 = mybir.dt.float32
F32R = mybir.dt.float32r
AF = mybir.ActivationFunctionType
ALU = mybir.AluOpType
AX = mybir.AxisListType


def _r(ap):
    return ap.bitcast(F32R)


def _bcast_inner(ap, n):
    """[p, m] AP -> [p, m, n] AP with innermost step 0 (free-dim broadcast)."""
    return bass.AP(tensor=ap.tensor, offset=ap.offset, ap=[*ap.ap, [0, n]])


import os


def build_nc(loop_n=1):
    nc = bacc.Bacc("TRN2", target_bir_lowering=False, debug=False)

    xf_d = nc.dram_tensor("xf", [C, N], F32, kind="ExternalInput")
    xh_d = nc.dram_tensor("xh", [C, NH], F32, kind="ExternalInput")
    wT_d = {t: nc.dram_tensor(f"w{t}T", [C, C], F32, kind="ExternalInput")
            for t in "qkvo"}
    brow_d = {"v": nc.dram_tensor("bv_row", [1, C], F32, kind="ExternalInput")}
    bcol_d = {t: nc.dram_tensor(f"b{t}_col", [C, 1], F32, kind="ExternalInput")
              for t in "qko"}
    gamma_d = nc.dram_tensor("gamma_c", [C, 1], F32, kind="ExternalInput")
    beta_d = nc.dram_tensor("beta_c", [C, 1], F32, kind="ExternalInput")
    g1_d = nc.dram_tensor("G1", [C, GROUPS], F32, kind="ExternalInput")
    g2_d = nc.dram_tensor("G2", [GROUPS, C], F32, kind="ExternalInput")
    ones_d = nc.dram_tensor("ones_row", [1, 512], F32, kind="ExternalInput")
    out_d = nc.dram_tensor("out", [C, NH], F32, kind="ExternalOutput")

    with tile.TileContext(nc) as tc:
        with (
            tc.tile_pool(name="persist", bufs=1) as pp,
            tc.tile_pool(name="mm", bufs=6, space="PSUM") as pmm,
            tc.tile_pool(name="opsum", bufs=2, space="PSUM") as pop,
            tc.tile_pool(name="epool", bufs=8) as pe_pool,
            tc.tile_pool(name="dpool", bufs=12) as pd_pool,
            tc.tile_pool(name="outpool", bufs=3) as pout,
        ):
            def ptile(shape, tag, dtype=F32):
                return pp.tile(shape, dtype, tag=tag, name=tag)

            def psum_t(tag_name):
                return pmm.tile([128, 512], F32, tag="mm", name=tag_name)

            import contextlib
            loop_ctx = (tc.For_i(0, loop_n, 1) if loop_n > 1
                        else contextlib.nullcontext())
            with loop_ctx:
                _body(nc, tc, ptile, psum_t, pe_pool, pd_pool, pout,
                      pop, xf_d, xh_d, wT_d, brow_d, bcol_d, gamma_d,
                      beta_d, g1_d, g2_d, ones_d, out_d)
    nc.compile()
    return nc


def _body(nc, tc, ptile, psum_t, pe_pool, pd_pool, pout, pop,
          xf_d, xh_d, wT_d, brow_d, bcol_d, gamma_d, beta_d, g1_d, g2_d,
          ones_d, out_d):
            # ---------------- loads ----------------
            xf = []
            xh = []
            wT = {t: [] for t in "qkvo"}
            gam, bet, g1 = [], [], []
            for i in range(2):
                t = ptile([128, N], f"xf{i}", F32R)
                for ch in range(4):
                    nc.sync.dma_start(
                        out=t[:, 1024 * ch:1024 * (ch + 1)],
                        in_=xf_d[128 * i:128 * (i + 1),
                                 1024 * ch:1024 * (ch + 1)].bitcast(F32R))
                xf.append(t)
                t = ptile([128, NH], f"xh{i}", F32R)
                for ch in range(2):
                    nc.sync.dma_start(
                        out=t[:, 1024 * ch:1024 * (ch + 1)],
                        in_=xh_d[128 * i:128 * (i + 1),
                                 1024 * ch:1024 * (ch + 1)].bitcast(F32R))
                xh.append(t)
                for w in "qkvo":
                    t = ptile([128, C], f"w{w}T{i}", F32R)
                    nc.sync.dma_start(out=t, in_=wT_d[w][128 * i:128 * (i + 1), :].bitcast(F32R))
                    wT[w].append(t)
                t = ptile([128, 1], f"gam{i}")
                nc.sync.dma_start(out=t, in_=gamma_d[128 * i:128 * (i + 1), :])
                gam.append(t)
                t = ptile([128, 1], f"bet{i}")
                nc.sync.dma_start(out=t, in_=beta_d[128 * i:128 * (i + 1), :])
                bet.append(t)
                t = ptile([128, GROUPS], f"g1_{i}")
                nc.sync.dma_start(out=t, in_=g1_d[128 * i:128 * (i + 1), :])
                g1.append(t)
            g2 = ptile([GROUPS, C], "g2")
            nc.sync.dma_start(out=g2, in_=g2_d[:, :])
            ones = ptile([1, 512], "ones", F32R)
            nc.sync.dma_start(out=ones, in_=ones_d[:, :].bitcast(F32R))
            brow = {}
            for w in "v":
                brow[w] = ptile([1, C], f"b{w}row", F32R)
                nc.sync.dma_start(out=brow[w], in_=brow_d[w][:, :].bitcast(F32R))
            bcol = {}
            for w in "qko":
                bcol[w] = []
                for i in range(2):
                    t = ptile([128, 1], f"b{w}col{i}")
                    nc.sync.dma_start(out=t, in_=bcol_d[w][128 * i:128 * (i + 1), :])
                    bcol[w].append(t)

            # ---------------- GroupNorm stats -> per-channel scale/shift ----
            NSUB = N // 512
            mstat = []
            for i in range(2):
                stats = pd_pool.tile([128, NSUB, 6], F32, tag="gnstats",
                                     name=f"gnstats{i}")
                for s in range(NSUB):
                    nc.vector.bn_stats(out=stats[:, s, :],
                                       in_=xf[i][:, 512 * s:512 * (s + 1)].bitcast(F32))
                mv = pd_pool.tile([128, 2], F32, tag="gnmv", name=f"gnmv{i}")
                nc.vector.bn_aggr(out=mv, in_=stats)
                ms = ptile([128, 2], f"mstat{i}")
                # ms = [mean_c, E[x^2]_c]
                nc.vector.tensor_mul(out=ms[:, 1:2], in0=mv[:, 0:1], in1=mv[:, 0:1])
                nc.vector.tensor_add(out=ms[:, 1:2], in0=ms[:, 1:2], in1=mv[:, 1:2])
                nc.vector.tensor_copy(out=ms[:, 0:1], in_=mv[:, 0:1])
                mstat.append(ms)

            pg_t = psum_t("pg")
            pg = pg_t[:GROUPS, :2]
            for i in range(2):
                nc.tensor.matmul(pg, lhsT=g1[i], rhs=mstat[i],
                                 start=(i == 0), stop=(i == 1))
            gstat = ptile([GROUPS, 2], "gstat")
            nc.vector.tensor_scalar_mul(out=gstat, in0=pg, scalar1=1.0 / 8.0)
            var32 = ptile([GROUPS, 1], "var32")
            nc.vector.tensor_mul(out=var32, in0=gstat[:, 0:1], in1=gstat[:, 0:1])
            nc.vector.tensor_sub(out=var32, in0=gstat[:, 1:2], in1=var32)
            std32 = ptile([GROUPS, 1], "std32")
            eps_t = ptile([GROUPS, 1], "eps_t")
            nc.vector.memset(eps_t, EPS)
            nc.scalar.activation(out=std32, in_=var32, func=AF.Sqrt, bias=eps_t)
            rstd = ptile([GROUPS, 1], "rstd")
            nc.vector.reciprocal(out=rstd, in_=std32)
            # one Newton polish of rsqrt: y <- y*(1.5 - 0.5*(var+eps)*y^2)
            tnr = ptile([GROUPS, 1], "tnr")
            nc.vector.tensor_mul(out=tnr, in0=rstd, in1=rstd)
            nc.vector.tensor_mul(out=tnr, in0=tnr, in1=var32)
            vepsy = ptile([GROUPS, 1], "vepsy")
            nc.vector.tensor_mul(out=vepsy, in0=rstd, in1=rstd)
            nc.vector.tensor_scalar_mul(out=vepsy, in0=vepsy, scalar1=EPS)
            nc.vector.tensor_add(out=tnr, in0=tnr, in1=vepsy)
            nc.vector.tensor_scalar_mul(out=tnr, in0=tnr, scalar1=-0.5)
            nc.vector.tensor_scalar_add(out=tnr, in0=tnr, scalar1=1.5)
            nc.vector.tensor_mul(out=rstd, in0=rstd, in1=tnr)

            grstat = ptile([GROUPS, 2], "grstat")
            nc.vector.tensor_copy(out=grstat[:, 0:1], in_=gstat[:, 0:1])
            nc.vector.tensor_copy(out=grstat[:, 1:2], in_=rstd)

            sc, sh = [], []
            for i in range(2):
                pc_t = psum_t(f"pc{i}")
                pc = pc_t[:128, :2]
                nc.tensor.matmul(pc, lhsT=g2[:, 128 * i:128 * (i + 1)],
                                 rhs=grstat, start=True, stop=True)
                s = ptile([128, 1], f"sc{i}")
                nc.vector.tensor_mul(out=s, in0=pc[:, 1:2], in1=gam[i])
                sc.append(s)
                h = ptile([128, 1], f"sh{i}", F32R)
                nc.vector.tensor_mul(out=h, in0=pc[:, 0:1], in1=s)
                nc.vector.tensor_sub(out=h, in0=bet[i], in1=h)
                sh.append(h)

            # effective v bias as a row (per-free-column bias for V^T)
            beffr = {}
            for w in "v":
                rp_t = psum_t(f"br{w}")
                rp = rp_t[:1, :C]
                for i in range(2):
                    nc.tensor.matmul(rp, lhsT=sh[i], rhs=wT[w][i],
                                     start=(i == 0), stop=(i == 1))
                bt = ptile([1, C], f"beff{w}", F32R)
                nc.vector.tensor_add(out=bt, in0=rp, in1=brow[w])
                beffr[w] = bt
            # effective q,k biases as columns (per-partition bias for ACT fuse)
            beffc = {}
            for w in "qk":
                beffc[w] = []
                for j in range(2):
                    bp_t = psum_t(f"bc{w}{j}")
                    bp = bp_t[:128, :1]
                    for i in range(2):
                        nc.tensor.matmul(bp,
                                         lhsT=wT[w][i][:, 128 * j:128 * (j + 1)].bitcast(F32),
                                         rhs=sh[i].bitcast(F32),
                                         start=(i == 0), stop=(i == 1))
                    t = ptile([128, 1], f"beffc{w}{j}")
                    nc.vector.tensor_add(out=t, in0=bp, in1=bcol[w][j])
                    beffc[w].append(t)

            # scale conv weights in place: WeffT[i,o] = wT[i,o] * scale_i
            for w in "qkv":
                for i in range(2):
                    nc.vector.tensor_scalar_mul(out=wT[w][i], in0=wT[w][i],
                                                scalar1=sc[i])

            # ---------------- convs: K, Q(half), V^T ----------------
            k_sb = [ptile([128, N], "k0", F32R), ptile([128, N], "k1", F32R)]
            q_sb = [ptile([128, NH], "q0", F32R), ptile([128, NH], "q1", F32R)]
            for j in range(2):
                for s in range(N // 512):
                    kp = psum_t(f"kp{j}_{s}")[:, :512]
                    for i in range(2):
                        nc.tensor.matmul(kp,
                                         lhsT=_r(wT["k"][i][:, 128 * j:128 * (j + 1)]),
                                         rhs=_r(xf[i][:, 512 * s:512 * (s + 1)]),
                                         start=(i == 0), stop=(i == 1))
                    nc.scalar.activation(out=k_sb[j][:, 512 * s:512 * (s + 1)],
                                         in_=kp, func=AF.Identity,
                                         bias=beffc["k"][j])
            for j in range(2):
                for s in range(NH // 512):
                    qp = psum_t(f"qp{j}_{s}")[:, :512]
                    for i in range(2):
                        nc.tensor.matmul(qp,
                                         lhsT=_r(wT["q"][i][:, 128 * j:128 * (j + 1)]),
                                         rhs=_r(xh[i][:, 512 * s:512 * (s + 1)]),
                                         start=(i == 0), stop=(i == 1))
                    nc.scalar.activation(out=q_sb[j][:, 512 * s:512 * (s + 1)],
                                         in_=qp, func=AF.Identity,
                                         bias=beffc["q"][j])

            vT = []
            for rt in range(N // 128):
                vp = psum_t(f"vp{rt}")[:, :C]
                for i in range(2):
                    nc.tensor.matmul(vp,
                                     lhsT=_r(xf[i][:, 128 * rt:128 * (rt + 1)]),
                                     rhs=_r(wT["v"][i]),
                                     start=(i == 0), stop=False)
                nc.tensor.matmul(vp, lhsT=_r(ones[:, :128]), rhs=_r(beffr["v"]),
                                 start=False, stop=True)
                t = ptile([128, C], f"vT{rt}", F32R)
                nc.vector.tensor_copy(out=t, in_=vp)
                vT.append(t)

            conv_only = os.environ.get("KSTAGE", "full") == "conv"
            if conv_only:
                for ct in range(2):
                    dbg = pout.tile([128, NH], F32, tag="dbg", name=f"dbg{ct}", bufs=2)
                    nc.vector.tensor_copy(out=dbg, in_=k_sb[ct][:, :NH].bitcast(F32))
                    nc.sync.dma_start(out=out_d[128 * ct:128 * (ct + 1), :], in_=dbg)
            if not conv_only:
              o_sb = [ptile([128, NH], "o_sb0", F32R), ptile([128, NH], "o_sb1", F32R)]

              # ---------------- attention ----------------
              for qt in range(4):
                  o_ps = [pop.tile([128, 512], F32, tag="o", name=f"ops{qt}_{ct}")
                          for ct in range(2)]
                  qcols = slice(512 * qt, 512 * (qt + 1))
                  for rt in range(N // 128):
                      e_t = pe_pool.tile([128, 512], F32R, tag="e",
                                         name=f"e{qt}_{rt}")
                      spx = psum_t(f"sp{qt}_{rt}")
                      for i in range(2):
                          nc.tensor.matmul(
                              spx,
                              lhsT=_r(k_sb[i][:, 128 * rt:128 * (rt + 1)]),
                              rhs=_r(q_sb[i][:, qcols]),
                              start=(i == 0), stop=(i == 1))
                      # E = exp(S/16); PSUM source, SBUF dest
                      nc.scalar.activation(out=e_t, in_=spx,
                                           func=AF.Exp, scale=1.0 / 16.0)
                      e3 = e_t.rearrange("p (w h) -> p w h", h=64)
                      d_t = pd_pool.tile([128, 8], F32, tag="d",
                                         name=f"d{qt}_{rt}")
                      nc.vector.tensor_reduce(out=d_t, in_=e3, axis=AX.X,
                                              op=ALU.add)
                      r_t = pd_pool.tile([128, 8], F32, tag="r",
                                         name=f"r{qt}_{rt}")
                      nc.vector.reciprocal(out=r_t, in_=d_t)
                      mul_eng = nc.vector if qt >= 1 and (rt + qt) % 4 == 0 else nc.gpsimd
                      mul_eng.tensor_mul(out=e3, in0=e3,
                                         in1=_bcast_inner(r_t, 64))
                      for ct in range(2):
                          nc.tensor.matmul(
                              o_ps[ct],
                              lhsT=_r(vT[rt][:, 128 * ct:128 * (ct + 1)]),
                              rhs=_r(e_t),
                              start=(rt == 0), stop=(rt == N // 128 - 1))
                  for ct in range(2):
                      nc.scalar.copy(out=o_sb[ct][:, qcols], in_=o_ps[ct])

                  # ---------------- out-proj + residual for this quarter -----
                  for ct in range(2):
                      prj = psum_t(f"prj{qt}_{ct}")
                      for i in range(2):
                          nc.tensor.matmul(
                              prj,
                              lhsT=_r(wT["o"][i][:, 128 * ct:128 * (ct + 1)]),
                              rhs=_r(o_sb[i][:, qcols]),
                              start=(i == 0), stop=(i == 1))
                      ot = pout.tile([128, 512], F32, tag="ot",
                                     name=f"ot{qt}_{ct}")
                      nc.vector.scalar_tensor_tensor(
                          out=ot, in0=prj, scalar=bcol["o"][ct],
                          in1=xh[ct][:, qcols].bitcast(F32),
                          op0=ALU.add, op1=ALU.add)
                      nc.sync.dma_start(out=out_d[128 * ct:128 * (ct + 1), qcols],
                                        in_=ot)


_NC = None


def _get_nc():
    global _NC
    if _NC is None:
        _NC = build_nc()
    return _NC


def _prep_in_maps(x, gamma, beta, q_w, q_b, k_w, k_b, v_w, v_b, o_w, o_b):
    x = np.ascontiguousarray(np.asarray(x, np.float32))
    g1 = np.zeros((C, GROUPS), np.float32)
    g1[np.arange(C), np.arange(C) // (C // GROUPS)] = 1.0
    shared = {
        "gamma_c": np.asarray(gamma, np.float32).reshape(C, 1).copy(),
        "beta_c": np.asarray(beta, np.float32).reshape(C, 1).copy(),
        "G1": g1,
        "G2": np.ascontiguousarray(g1.T),
        "ones_row": np.ones((1, 512), np.float32),
    }
    for t, wm, bv in (("q", q_w, q_b), ("k", k_w, k_b),
                      ("v", v_w, v_b), ("o", o_w, o_b)):
        shared[f"w{t}T"] = np.ascontiguousarray(np.asarray(wm, np.float32).T)
        if t == "v":
            shared["bv_row"] = np.asarray(bv, np.float32).reshape(1, C).copy()
        else:
            shared[f"b{t}_col"] = np.asarray(bv, np.float32).reshape(C, 1).copy()
    in_maps = []
    for core in range(8):
        b, half = core // 2, core % 2
        xb = x[b].reshape(C, N)
        xh = np.ascontiguousarray(
            x[b][:, :, half * WH:(half + 1) * WH].transpose(0, 2, 1)
        ).reshape(C, NH)
        in_maps.append(dict(shared, xf=np.ascontiguousarray(xb), xh=xh))
    return in_maps


def _unshard_out(per_core_out):
    out = np.empty((B, C, H, W), np.float32)
    for core in range(8):
        b, half = core // 2, core % 2
        oh = per_core_out[core].reshape(C, WH, H).transpose(0, 2, 1)
        out[b][:, :, half * WH:(half + 1) * WH] = oh
    return out


def run(trace=False, **inputs):
    in_maps = _prep_in_maps(**inputs)
    nc = _get_nc()
    res = run_bass_kernel_spmd(nc, in_maps, core_ids=list(range(8)), trace=trace)
    out = _unshard_out([res.results[core]["out"] for core in range(8)])
    return out, res


def kernel(**inputs):
    out, _ = run(trace=False, **inputs)
    return out



# revision 6
# speedup vs baseline: 61.7989x; 2.3381x over previous
"""AttnBlock (GroupNorm + 1x1-conv QKV + spatial attention w/ softmax over
query-h + out-proj + residual) for Trainium2, 8 NeuronCores.

Sharding: core = 2*b + w_half  (4 samples x 2 halves of the w axis).
Attention here softmax-normalizes over the h index of the *query* location,
so for a fixed w column the 64 h-values form one softmax group; splitting the
spatial grid by w keeps every softmax group on a single core.

Device layout notes (per core):
  - spatial index packed w-major: p = w'*64 + h   (w' in [0,32) local half)
  - xf  [256,4096]: full sample, natural (c, h*64+w) layout (GN stats, K, V)
  - xh  [256,2048]: this half's columns, w-major packed (Q, residual)
  - S^T tile [r_keys=128, p=1024] = K^T Q, exp on ScalarE (scale=1/16 folded),
    segmented sum over h (contiguous 64-blocks) on VectorE, reciprocal,
    broadcast-multiply -> attn, then O = V^T-weighted sum via PE accumulation.
  - GroupNorm is folded into the q/k/v conv weights on device:
    xn = scale_c * x + shift_c  =>  Weff = W*diag(scale), beff = W@shift + b.
  - All big matmuls use float32r (FP22 multiply, fp32 accumulate): 1 cycle/row
    on the PE when the moving dim >= 256 (plain fp32 is 4 cycles/row).
"""

import numpy as np

import concourse.bass as bass
import concourse.bacc as bacc
import concourse.mybir as mybir
import concourse.tile as tile
from concourse.bass_utils import run_bass_kernel_spmd

B, C, H, W = 4, 256, 64, 64
N = H * W            # 4096 spatial
NH = N // 2          # 2048 per w-half
WH = W // 2          # 32 local w' values
GROUPS = 32
EPS = 1e-5
F32 = mybir.dt.float32
F32R = mybir.dt.float32r
AF = mybir.ActivationFunctionType
ALU = mybir.AluOpType
AX = mybir.AxisListType


def _r(ap):
    return ap.bitcast(F32R)


def _bcast_inner(ap, n):
    """[p, m] AP -> [p, m, n] AP with innermost step 0 (free-dim broadcast)."""
    return bass.AP(tensor=ap.tensor, offset=ap.offset, ap=[*ap.ap, [0, n]])


import os


def build_nc(loop_n=1):
    nc = bacc.Bacc("TRN2", target_bir_lowering=False, debug=False)

    xf_d = nc.dram_tensor("xf", [C, N], F32, kind="ExternalInput")
    xh_d = nc.dram_tensor("xh", [C, NH], F32, kind="ExternalInput")
    wT_d = {t: nc.dram_tensor(f"w{t}T", [C, C], F32, kind="ExternalInput")
            for t in "qkvo"}
    brow_d = {"v": nc.dram_tensor("bv_row", [1, C], F32, kind="ExternalInput")}
    bcol_d = {t: nc.dram_tensor(f"b{t}_col", [C, 1], F32, kind="ExternalInput")
              for t in "qko"}
    gamma_d = nc.dram_tensor("gamma_c", [C, 1], F32, kind="ExternalInput")
    beta_d = nc.dram_tensor("beta_c", [C, 1], F32, kind="ExternalInput")
    g1_d = nc.dram_tensor("G1", [C, GROUPS], F32, kind="ExternalInput")
    g2_d = nc.dram_tensor("G2", [GROUPS, C], F32, kind="ExternalInput")
    ones_d = nc.dram_tensor("ones_row", [1, 512], F32, kind="ExternalInput")
    out_d = nc.dram_tensor("out", [C, NH], F32, kind="ExternalOutput")

    with tile.TileContext(nc) as tc:
        with (
            tc.tile_pool(name="persist", bufs=1) as pp,
            tc.tile_pool(name="mm", bufs=6, space="PSUM") as pmm,
            tc.tile_pool(name="opsum", bufs=2, space="PSUM") as pop,
            tc.tile_pool(name="epool", bufs=8) as pe_pool,
            tc.tile_pool(name="dpool", bufs=12) as pd_pool,
            tc.tile_pool(name="outpool", bufs=3) as pout,
        ):
            def ptile(shape, tag, dtype=F32):
                return pp.tile(shape, dtype, tag=tag, name=tag)

            def psum_t(tag_name):
                return pmm.tile([128, 512], F32, tag="mm", name=tag_name)

            import contextlib
            loop_ctx = (tc.For_i(0, loop_n, 1) if loop_n > 1
                        else contextlib.nullcontext())
            with loop_ctx:
                _body(nc, tc, ptile, psum_t, pe_pool, pd_pool, pout,
                      pop, xf_d, xh_d, wT_d, brow_d, bcol_d, gamma_d,
                      beta_d, g1_d, g2_d, ones_d, out_d)
    nc.compile()
    return nc


def _body(nc, tc, ptile, psum_t, pe_pool, pd_pool, pout, pop,
          xf_d, xh_d, wT_d, brow_d, bcol_d, gamma_d, beta_d, g1_d, g2_d,
          ones_d, out_d):
            # ---------------- loads ----------------
            xf = []
            xh = []
            wT = {t: [] for t in "qkvo"}
            gam, bet, g1 = [], [], []
            for i in range(2):
                t = ptile([128, N], f"xf{i}", F32R)
                for ch in range(4):
                    nc.sync.dma_start(
                        out=t[:, 1024 * ch:1024 * (ch + 1)],
                        in_=xf_d[128 * i:128 * (i + 1),
                                 1024 * ch:1024 * (ch + 1)].bitcast(F32R))
                xf.append(t)
                t = ptile([128, NH], f"xh{i}", F32R)
                for ch in range(2):
                    nc.sync.dma_start(
                        out=t[:, 1024 * ch:1024 * (ch + 1)],
                        in_=xh_d[128 * i:128 * (i + 1),
                                 1024 * ch:1024 * (ch + 1)].bitcast(F32R))
                xh.append(t)
                for w in "qkvo":
                    t = ptile([128, C], f"w{w}T{i}", F32R)
                    nc.sync.dma_start(out=t, in_=wT_d[w][128 * i:128 * (i + 1), :].bitcast(F32R))
                    wT[w].append(t)
                t = ptile([128, 1], f"gam{i}")
                nc.sync.dma_start(out=t, in_=gamma_d[128 * i:128 * (i + 1), :])
                gam.append(t)
                t = ptile([128, 1], f"bet{i}")
                nc.sync.dma_start(out=t, in_=beta_d[128 * i:128 * (i + 1), :])
                bet.append(t)
                t = ptile([128, GROUPS], f"g1_{i}")
                nc.sync.dma_start(out=t, in_=g1_d[128 * i:128 * (i + 1), :])
                g1.append(t)
            g2 = ptile([GROUPS, C], "g2")
            nc.sync.dma_start(out=g2, in_=g2_d[:, :])
            ones = ptile([1, 512], "ones", F32R)
            nc.sync.dma_start(out=ones, in_=ones_d[:, :].bitcast(F32R))
            brow = {}
            for w in "v":
                brow[w] = ptile([1, C], f"b{w}row", F32R)
                nc.sync.dma_start(out=brow[w], in_=brow_d[w][:, :].bitcast(F32R))
            bcol = {}
            for w in "qko":
                bcol[w] = []
                for i in range(2):
                    t = ptile([128, 1], f"b{w}col{i}")
                    nc.sync.dma_start(out=t, in_=bcol_d[w][128 * i:128 * (i + 1), :])
                    bcol[w].append(t)

            # PE warmup: tiny matmuls chained to each arriving DMA chunk keep
            # the PE active through the load phase so the HAM clock-gate is at
            # 8/8 when the convs start (it re-throttles after ~3.4us idle).
            warm_ps = psum_t("warm")[:1, :512]
            for i in range(2):
                for ch in range(4):
                    nc.tensor.matmul(
                        warm_ps,
                        lhsT=_r(xf[i][:, 1024 * ch:1024 * ch + 1]),
                        rhs=_r(xf[i][:, 1024 * ch:1024 * ch + 512]),
                        start=True, stop=True)
                for ch in range(2):
                    nc.tensor.matmul(
                        warm_ps,
                        lhsT=_r(xh[i][:, 1024 * ch:1024 * ch + 1]),
                        rhs=_r(xh[i][:, 1024 * ch:1024 * ch + 512]),
                        start=True, stop=True)

            # ---------------- GroupNorm stats -> per-channel scale/shift ----
            NSUB = N // 512
            mstat = []
            for i in range(2):
                stats = pd_pool.tile([128, NSUB, 6], F32, tag="gnstats",
                                     name=f"gnstats{i}")
                for s in range(NSUB):
                    nc.vector.bn_stats(out=stats[:, s, :],
                                       in_=xf[i][:, 512 * s:512 * (s + 1)].bitcast(F32))
                mv = pd_pool.tile([128, 2], F32, tag="gnmv", name=f"gnmv{i}")
                nc.vector.bn_aggr(out=mv, in_=stats)
                ms = ptile([128, 2], f"mstat{i}")
                # ms = [mean_c, E[x^2]_c]
                nc.vector.tensor_mul(out=ms[:, 1:2], in0=mv[:, 0:1], in1=mv[:, 0:1])
                nc.vector.tensor_add(out=ms[:, 1:2], in0=ms[:, 1:2], in1=mv[:, 1:2])
                nc.vector.tensor_copy(out=ms[:, 0:1], in_=mv[:, 0:1])
                mstat.append(ms)

            pg_t = psum_t("pg")
            pg = pg_t[:GROUPS, :2]
            for i in range(2):
                nc.tensor.matmul(pg, lhsT=g1[i], rhs=mstat[i],
                                 start=(i == 0), stop=(i == 1))
            gstat = ptile([GROUPS, 2], "gstat")
            nc.vector.tensor_scalar_mul(out=gstat, in0=pg, scalar1=1.0 / 8.0)
            var32 = ptile([GROUPS, 1], "var32")
            nc.vector.tensor_mul(out=var32, in0=gstat[:, 0:1], in1=gstat[:, 0:1])
            nc.vector.tensor_sub(out=var32, in0=gstat[:, 1:2], in1=var32)
            std32 = ptile([GROUPS, 1], "std32")
            eps_t = ptile([GROUPS, 1], "eps_t")
            nc.vector.memset(eps_t, EPS)
            nc.scalar.activation(out=std32, in_=var32, func=AF.Sqrt, bias=eps_t)
            rstd = ptile([GROUPS, 1], "rstd")
            nc.vector.reciprocal(out=rstd, in_=std32)
            # one Newton polish of rsqrt: y <- y*(1.5 - 0.5*(var+eps)*y^2)
            tnr = ptile([GROUPS, 1], "tnr")
            nc.vector.tensor_mul(out=tnr, in0=rstd, in1=rstd)
            nc.vector.tensor_mul(out=tnr, in0=tnr, in1=var32)
            vepsy = ptile([GROUPS, 1], "vepsy")
            nc.vector.tensor_mul(out=vepsy, in0=rstd, in1=rstd)
            nc.vector.tensor_scalar_mul(out=vepsy, in0=vepsy, scalar1=EPS)
            nc.vector.tensor_add(out=tnr, in0=tnr, in1=vepsy)
            nc.vector.tensor_scalar_mul(out=tnr, in0=tnr, scalar1=-0.5)
            nc.vector.tensor_scalar_add(out=tnr, in0=tnr, scalar1=1.5)
            nc.vector.tensor_mul(out=rstd, in0=rstd, in1=tnr)

            grstat = ptile([GROUPS, 2], "grstat")
            nc.vector.tensor_copy(out=grstat[:, 0:1], in_=gstat[:, 0:1])
            nc.vector.tensor_copy(out=grstat[:, 1:2], in_=rstd)

            sc, sh = [], []
            for i in range(2):
                pc_t = psum_t(f"pc{i}")
                pc = pc_t[:128, :2]
                nc.tensor.matmul(pc, lhsT=g2[:, 128 * i:128 * (i + 1)],
                                 rhs=grstat, start=True, stop=True)
                s = ptile([128, 1], f"sc{i}")
                nc.vector.tensor_mul(out=s, in0=pc[:, 1:2], in1=gam[i])
                sc.append(s)
                h = ptile([128, 1], f"sh{i}", F32R)
                nc.vector.tensor_mul(out=h, in0=pc[:, 0:1], in1=s)
                nc.vector.tensor_sub(out=h, in0=bet[i], in1=h)
                sh.append(h)

            # effective v bias as a row (per-free-column bias for V^T)
            beffr = {}
            for w in "v":
                rp_t = psum_t(f"br{w}")
                rp = rp_t[:1, :C]
                for i in range(2):
                    nc.tensor.matmul(rp, lhsT=sh[i], rhs=wT[w][i],
                                     start=(i == 0), stop=(i == 1))
                bt = ptile([1, C], f"beff{w}", F32R)
                nc.vector.tensor_add(out=bt, in0=rp, in1=brow[w])
                beffr[w] = bt
            # effective q,k biases as columns (per-partition bias for ACT fuse)
            beffc = {}
            for w in "qk":
                beffc[w] = []
                for j in range(2):
                    bp_t = psum_t(f"bc{w}{j}")
                    bp = bp_t[:128, :1]
                    for i in range(2):
                        nc.tensor.matmul(bp,
                                         lhsT=wT[w][i][:, 128 * j:128 * (j + 1)].bitcast(F32),
                                         rhs=sh[i].bitcast(F32),
                                         start=(i == 0), stop=(i == 1))
                    t = ptile([128, 1], f"beffc{w}{j}")
                    nc.vector.tensor_add(out=t, in0=bp, in1=bcol[w][j])
                    beffc[w].append(t)

            # scale conv weights in place: WeffT[i,o] = wT[i,o] * scale_i
            for w in "qkv":
                for i in range(2):
                    nc.vector.tensor_scalar_mul(out=wT[w][i], in0=wT[w][i],
                                                scalar1=sc[i])

            # ---------------- convs: K, Q(half), V^T ----------------
            k_sb = [ptile([128, N], "k0", F32R), ptile([128, N], "k1", F32R)]
            q_sb = [ptile([128, NH], "q0", F32R), ptile([128, NH], "q1", F32R)]
            for j in range(2):
                for s in range(N // 512):
                    kp = psum_t(f"kp{j}_{s}")[:, :512]
                    for i in range(2):
                        nc.tensor.matmul(kp,
                                         lhsT=_r(wT["k"][i][:, 128 * j:128 * (j + 1)]),
                                         rhs=_r(xf[i][:, 512 * s:512 * (s + 1)]),
                                         start=(i == 0), stop=(i == 1))
                    nc.scalar.activation(out=k_sb[j][:, 512 * s:512 * (s + 1)],
                                         in_=kp, func=AF.Identity,
                                         bias=beffc["k"][j])
            for j in range(2):
                for s in range(NH // 512):
                    qp = psum_t(f"qp{j}_{s}")[:, :512]
                    for i in range(2):
                        nc.tensor.matmul(qp,
                                         lhsT=_r(wT["q"][i][:, 128 * j:128 * (j + 1)]),
                                         rhs=_r(xh[i][:, 512 * s:512 * (s + 1)]),
                                         start=(i == 0), stop=(i == 1))
                    nc.scalar.activation(out=q_sb[j][:, 512 * s:512 * (s + 1)],
                                         in_=qp, func=AF.Identity,
                                         bias=beffc["q"][j])

            vT = []
            for rt in range(N // 128):
                vp = psum_t(f"vp{rt}")[:, :C]
                for i in range(2):
                    nc.tensor.matmul(vp,
                                     lhsT=_r(xf[i][:, 128 * rt:128 * (rt + 1)]),
                                     rhs=_r(wT["v"][i]),
                                     start=(i == 0), stop=False)
                nc.tensor.matmul(vp, lhsT=_r(ones[:, :128]), rhs=_r(beffr["v"]),
                                 start=False, stop=True)
                t = ptile([128, C], f"vT{rt}", F32R)
                nc.vector.tensor_copy(out=t, in_=vp)
                vT.append(t)

            conv_only = os.environ.get("KSTAGE", "full") == "conv"
            if conv_only:
                for ct in range(2):
                    dbg = pout.tile([128, NH], F32, tag="dbg", name=f"dbg{ct}", bufs=2)
                    nc.vector.tensor_copy(out=dbg, in_=k_sb[ct][:, :NH].bitcast(F32))
                    nc.sync.dma_start(out=out_d[128 * ct:128 * (ct + 1), :], in_=dbg)
            if not conv_only:
              o_sb = [ptile([128, NH], "o_sb0", F32R), ptile([128, NH], "o_sb1", F32R)]

              # ---------------- attention ----------------
              for qt in range(4):
                  o_ps = [pop.tile([128, 512], F32, tag="o", name=f"ops{qt}_{ct}")
                          for ct in range(2)]
                  qcols = slice(512 * qt, 512 * (qt + 1))
                  for rt in range(N // 128):
                      e_t = pe_pool.tile([128, 512], F32R, tag="e",
                                         name=f"e{qt}_{rt}")
                      spx = psum_t(f"sp{qt}_{rt}")
                      for i in range(2):
                          nc.tensor.matmul(
                              spx,
                              lhsT=_r(k_sb[i][:, 128 * rt:128 * (rt + 1)]),
                              rhs=_r(q_sb[i][:, qcols]),
                              start=(i == 0), stop=(i == 1))
                      # E = exp(S/16); PSUM source, SBUF dest
                      nc.scalar.activation(out=e_t, in_=spx,
                                           func=AF.Exp, scale=1.0 / 16.0)
                      e3 = e_t.rearrange("p (w h) -> p w h", h=64)
                      d_t = pd_pool.tile([128, 8], F32, tag="d",
                                         name=f"d{qt}_{rt}")
                      nc.vector.tensor_reduce(out=d_t, in_=e3, axis=AX.X,
                                              op=ALU.add)
                      r_t = pd_pool.tile([128, 8], F32, tag="r",
                                         name=f"r{qt}_{rt}")
                      nc.vector.reciprocal(out=r_t, in_=d_t)
                      mul_eng = nc.vector if qt >= 1 and (rt + qt) % 4 == 0 else nc.gpsimd
                      mul_eng.tensor_mul(out=e3, in0=e3,
                                         in1=_bcast_inner(r_t, 64))
                      for ct in range(2):
                          nc.tensor.matmul(
                              o_ps[ct],
                              lhsT=_r(vT[rt][:, 128 * ct:128 * (ct + 1)]),
                              rhs=_r(e_t),
                              start=(rt == 0), stop=(rt == N // 128 - 1))
                  for ct in range(2):
                      nc.scalar.copy(out=o_sb[ct][:, qcols], in_=o_ps[ct])

                  # ---------------- out-proj + residual for this quarter -----
                  for ct in range(2):
                      prj = psum_t(f"prj{qt}_{ct}")
                      for i in range(2):
                          nc.tensor.matmul(
                              prj,
                              lhsT=_r(wT["o"][i][:, 128 * ct:128 * (ct + 1)]),
                              rhs=_r(o_sb[i][:, qcols]),
                              start=(i == 0), stop=(i == 1))
                      ot = pout.tile([128, 512], F32, tag="ot",
                                     name=f"ot{qt}_{ct}")
                      nc.vector.scalar_tensor_tensor(
                          out=ot, in0=prj, scalar=bcol["o"][ct],
                          in1=xh[ct][:, qcols].bitcast(F32),
                          op0=ALU.add, op1=ALU.add)
                      nc.sync.dma_start(out=out_d[128 * ct:128 * (ct + 1), qcols],
                                        in_=ot)


_NC = None


def _get_nc():
    global _NC
    if _NC is None:
        _NC = build_nc()
    return _NC


def _prep_in_maps(x, gamma, beta, q_w, q_b, k_w, k_b, v_w, v_b, o_w, o_b):
    x = np.ascontiguousarray(np.asarray(x, np.float32))
    g1 = np.zeros((C, GROUPS), np.float32)
    g1[np.arange(C), np.arange(C) // (C // GROUPS)] = 1.0
    shared = {
        "gamma_c": np.asarray(gamma, np.float32).reshape(C, 1).copy(),
        "beta_c": np.asarray(beta, np.float32).reshape(C, 1).copy(),
        "G1": g1,
        "G2": np.ascontiguousarray(g1.T),
        "ones_row": np.ones((1, 512), np.float32),
    }
    for t, wm, bv in (("q", q_w, q_b), ("k", k_w, k_b),
                      ("v", v_w, v_b), ("o", o_w, o_b)):
        shared[f"w{t}T"] = np.ascontiguousarray(np.asarray(wm, np.float32).T)
        if t == "v":
            shared["bv_row"] = np.asarray(bv, np.float32).reshape(1, C).copy()
        else:
            shared[f"b{t}_col"] = np.asarray(bv, np.float32).reshape(C, 1).copy()
    in_maps = []
    for core in range(8):
        b, half = core // 2, core % 2
        xb = x[b].reshape(C, N)
        xh = np.ascontiguousarray(
            x[b][:, :, half * WH:(half + 1) * WH].transpose(0, 2, 1)
        ).reshape(C, NH)
        in_maps.append(dict(shared, xf=np.ascontiguousarray(xb), xh=xh))
    return in_maps


def _unshard_out(per_core_out):
    out = np.empty((B, C, H, W), np.float32)
    for core in range(8):
        b, half = core // 2, core % 2
        oh = per_core_out[core].reshape(C, WH, H).transpose(0, 2, 1)
        out[b][:, :, half * WH:(half + 1) * WH] = oh
    return out


def run(trace=False, **inputs):
    in_maps = _prep_in_maps(**inputs)
    nc = _get_nc()
    res = run_bass_kernel_spmd(nc, in_maps, core_ids=list(range(8)), trace=trace)
    out = _unshard_out([res.results[core]["out"] for core in range(8)])
    return out, res


def kernel(**inputs):
    out, _ = run(trace=False, **inputs)
    return out

